# revision 5
# baseline (speedup 1.0000x reference)
"""GNN message-passing kernel for 8 Trainium2 NeuronCores (Bass/Tile).

Sharding: each core owns 2500 nodes + all edges targeting them. Node state
is feature-major in SBUF; after each GRU update it is written node-major
(bf16, 128-wide rows) to DRAM and AllGathered so any core can dma_gather
arbitrary source rows.  Per-edge weights are never materialized:
  msg_e = h[src_e] @ (ea_e @ nnW^T).reshape(D,D)
is computed as  Z[(k,i),e] = ea[k,e] * h[i,src_e];  msg = G^T @ Z
with G a host-prepacked rearrangement of nnW.  segment-sum over targets is
a matmul against host-built 0/1 staircase blocks (edges sorted by target,
each 512-node tile padded to an integral number of 128-edge chunks); 1/deg
is applied per-edge (exact fp32) on the scalar engine during PSUM evac.
"""
import sys
sys.path.insert(0, "/opt/trn_rl_repo")
import numpy as np
import ml_dtypes

import concourse.bass as bass
import concourse.bacc as bacc
import concourse.mybir as mybir
import concourse.tile as tile
from concourse.bass_utils import run_bass_kernel_spmd

F32 = mybir.dt.float32
BF16 = mybir.dt.bfloat16
I16 = mybir.dt.int16
AF = mybir.ActivationFunctionType
ALU = mybir.AluOpType

N, E, E3, D = 20000, 30000, 4000, 64
D2 = 2 * D
C = 8
NL = N // C          # nodes per core (2500)
NLP = 2560           # padded rows per core in gathered state (mult of 128)
NT = 512             # node-tile / matmul moving chunk
NTC = (NL + NT - 1) // NT
SLOPE = (1.0 / 8.0 + 1.0 / 3.0) / 2.0
EPS = 1e-5
EF = 512             # final readout edges per core (500 real)

bfd = ml_dtypes.bfloat16


# ----------------------------------------------------------------- host prep

def _wrap16(idx):
    n = len(idx)
    w = idx.reshape(n // 16, 16).T.astype(np.int16)
    return np.tile(w, (8, 1)).copy()


def _pad_id(n):
    """global node id -> padded row id in gathered state."""
    return (n // NL) * NLP + (n % NL)


def _affine_bn(g, be, m, v):
    a = g / np.sqrt(v + EPS)
    return a, be - m * a


def _prep_edges(src, tgt, attr, n_attr):
    owner = tgt // NL
    per_core = []
    maxrun = 1
    for c in range(C):
        sel = np.where(owner == c)[0]
        tl = tgt[sel] - c * NL
        order = np.argsort(tl, kind="stable")
        sel, tl = sel[order], tl[order]
        per_core.append((sel, tl))
        for t in range(NTC):
            maxrun = max(maxrun, int(((tl // NT) == t).sum()))
    cpt = (maxrun + 127) // 128
    ep = NTC * cpt * 128

    gidx = np.zeros((C, ep), np.int64)
    eaT = np.zeros((C, n_attr, ep), np.float32)
    invdeg_e = np.zeros((C, ep), np.float32)
    s_blocks = np.zeros((C, NTC * cpt, 128, NT), bfd)
    deg = np.maximum(np.bincount(tgt, minlength=N), 1).astype(np.float32)

    for c, (sel, tl) in enumerate(per_core):
        for t in range(NTC):
            msk = (tl // NT) == t
            idxs, tls = sel[msk], tl[msk]
            k = len(idxs)
            pos = t * cpt * 128
            gidx[c, pos:pos + k] = _pad_id(src[idxs])
            eaT[c, :, pos:pos + k] = attr[idxs].T
            invdeg_e[c, pos:pos + k] = 1.0 / deg[c * NL + tls]
            rel = tls - t * NT
            ar = np.arange(k) + pos
            s_blocks[c, ar // 128, ar % 128, rel] = 1.0
    return dict(ep=ep, gidx=gidx, eaT=eaT, invdeg_e=invdeg_e, s=s_blocks)


def _host_prep(inp):
    g = lambda k: np.asarray(inp[k], np.float32)
    ei = np.asarray(inp["edge_index"], np.int64)
    ei3 = np.asarray(inp["edge_index3"], np.int64)

    a, b = _affine_bn(g("nx_g"), g("nx_be"), g("nx_m"), g("nx_v"))
    Wln = (a[:, None] * g("ln_W").T).astype(np.float32)
    bln = (b @ g("ln_W").T + g("ln_b")).astype(np.float32)

    e1 = _prep_edges(ei[0], ei[1], g("edge_attr"), 19)
    nn1 = g("nn1_W")
    G1 = np.zeros((128, 6, 64), np.float32)           # partition-first
    REP1 = np.zeros((12, 6, 128), np.float32)
    for cc in range(6):
        for half, k in enumerate((2 * cc, 2 * cc + 1)):
            G1[half * 64:(half + 1) * 64, cc, :] = nn1[:, k].reshape(64, 64)
            REP1[k, cc, half * 64:(half + 1) * 64] = 1.0

    src3 = np.concatenate([ei3[0], ei3[1]])
    tgt3 = np.concatenate([ei3[1], ei3[0]])
    attr3 = np.concatenate([g("edge_attr3"), g("edge_attr3")], axis=0)
    e2 = _prep_edges(src3, tgt3, attr3, 8)
    nn2 = g("nn2_W")
    G2 = np.zeros((128, 8, 128), np.float32)
    REP2 = np.zeros((8, 8, 128), np.float32)
    for k in range(8):
        G2[:, k, :] = nn2[:, k].reshape(D2, D2)
        REP2[k, k, :] = 1.0

    f_i0 = np.zeros((C, EF), np.int64)
    f_i1 = np.zeros((C, EF), np.int64)
    ea3locT = np.zeros((C, 8, EF), np.float32)
    npc = E3 // C
    for c in range(C):
        lo = c * npc
        f_i0[c, :npc] = _pad_id(ei3[0, lo:lo + npc])
        f_i1[c, :npc] = _pad_id(ei3[1, lo:lo + npc])
        ea3locT[c, :, :npc] = g("edge_attr3")[lo:lo + npc].T

    a_nm, b_nm = _affine_bn(g("nm_g"), g("nm_be"), g("nm_m"), g("nm_v"))
    a_nm = a_nm.copy()
    a_nm[0:D2] *= 0.5
    lwWt = (g("lw_W") * a_nm[:, None]).T.astype(np.float32)   # (8,384)
    lbp = (g("lb_W")[0] + b_nm @ g("lw_W")).astype(np.float32)

    alc, blc = _affine_bn(g("lc_g"), g("lc_be"), g("lc_m"), g("lc_v"))
    W1c = g("lc_w1") * alc[None, :]
    b1c = (g("lc_w1") @ blc + g("lc_b1")).astype(np.float32)

    bih1, bhh1 = g("g1_bih"), g("g1_bhh")
    bih2, bhh2 = g("g2_bih"), g("g2_bhh")

    xs = g("x")
    in_maps = []
    for c in range(C):
        m = {
            "xT": xs[c * NL:(c + 1) * NL].T,
            "eaT1": e1["eaT"][c],
            "gidx1": _wrap16(e1["gidx"][c]),
            "invd1": e1["invdeg_e"][c].reshape(-1, 128).T,
            "S1": e1["s"][c],
            "eaT3": e2["eaT"][c],
            "gidx3": _wrap16(e2["gidx"][c]),
            "invd3": e2["invdeg_e"][c].reshape(-1, 128).T,
            "S3": e2["s"][c],
            "gf0": _wrap16(f_i0[c]), "gf1": _wrap16(f_i1[c]),
            "ea3locT": ea3locT[c],
            "Wln": Wln, "bln": bln.reshape(-1, 1),
            "leWt": g("le_W").T, "leb": g("le_b").reshape(-1, 1),
            "G1": G1.astype(bfd), "REP1": REP1.astype(bfd),
            "G2": G2.astype(bfd), "REP2": REP2.astype(bfd),
            "c1b": g("c1_b").reshape(-1, 1), "c2b": g("c2_b").reshape(-1, 1),
            "wih1": g("g1_wih").T.astype(bfd), "whh1": g("g1_whh").T.astype(bfd),
            "br1": (bih1 + bhh1)[0:D].reshape(-1, 1),
            "bz1": (bih1 + bhh1)[D:2 * D].reshape(-1, 1),
            "bin1": bih1[2 * D:].reshape(-1, 1),
            "bhn1": bhh1[2 * D:].reshape(-1, 1),
            "wih2": g("g2_wih").T.astype(bfd), "whh2": g("g2_whh").T.astype(bfd),
            "br2": (bih2 + bhh2)[0:D2].reshape(-1, 1),
            "bz2": (bih2 + bhh2)[D2:2 * D2].reshape(-1, 1),
            "bin2": bih2[2 * D2:].reshape(-1, 1),
            "bhn2": bhh2[2 * D2:].reshape(-1, 1),
            "W1cT": W1c.T.astype(bfd), "b1c": b1c.reshape(-1, 1),
            "W2cT": g("lc_w2").T.astype(bfd), "b2c": g("lc_b2").reshape(-1, 1),
            "lwWt": lwWt, "lbp": lbp.reshape(-1, 1),
            "eye": np.eye(128, dtype=bfd),
        }
        in_maps.append({k: np.ascontiguousarray(v) for k, v in m.items()})
    static = (e1["ep"], e2["ep"])
    return static, in_maps


# ------------------------------------------------------------- kernel builder

def _build(EP1, EP3):
    nc = bacc.Bacc("TRN2", target_bir_lowering=False, debug=False,
                   num_devices=C)
    J1, J3 = EP1 // 128, EP3 // 128
    LZ = max(6 * ((EP1 // 128 + 1) // 2 + 1) * 128, 8 * EP3)  # z arena
    LE = max(6 * EP1, 8 * EP3)            # eax arena
    LG = max(EP1, EP3)
    LM = max(J1 * 64, J3 * 128)           # msg_em arena

    def inp(name, shape, dt=F32):
        return nc.dram_tensor(name, list(shape), dt, kind="ExternalInput")

    xT = inp("xT", (8, NL))
    eaT1 = inp("eaT1", (19, EP1)); gidx1 = inp("gidx1", (128, EP1 // 16), I16)
    invd1 = inp("invd1", (128, J1)); S1 = inp("S1", (J1, 128, NT), BF16)
    eaT3 = inp("eaT3", (8, EP3)); gidx3 = inp("gidx3", (128, EP3 // 16), I16)
    invd3 = inp("invd3", (128, J3)); S3 = inp("S3", (J3, 128, NT), BF16)
    gf0 = inp("gf0", (128, EF // 16), I16); gf1 = inp("gf1", (128, EF // 16), I16)
    ea3locT = inp("ea3locT", (8, EF))
    Wln = inp("Wln", (8, 64)); bln = inp("bln", (64, 1))
    leWt = inp("leWt", (19, 12)); leb = inp("leb", (12, 1))
    G1 = inp("G1", (128, 6, 64), BF16); REP1 = inp("REP1", (12, 6, 128), BF16)
    G2 = inp("G2", (128, 8, 128), BF16); REP2 = inp("REP2", (8, 8, 128), BF16)
    c1b = inp("c1b", (64, 1)); c2b = inp("c2b", (128, 1))
    wih1 = inp("wih1", (64, 192), BF16); whh1 = inp("whh1", (64, 192), BF16)
    br1 = inp("br1", (64, 1)); bz1 = inp("bz1", (64, 1))
    bin1 = inp("bin1", (64, 1)); bhn1 = inp("bhn1", (64, 1))
    wih2 = inp("wih2", (128, 384), BF16); whh2 = inp("whh2", (128, 384), BF16)
    br2 = inp("br2", (128, 1)); bz2 = inp("bz2", (128, 1))
    bin2 = inp("bin2", (128, 1)); bhn2 = inp("bhn2", (128, 1))
    W1cT = inp("W1cT", (64, 128), BF16); b1c = inp("b1c", (128, 1))
    W2cT = inp("W2cT", (128, 128), BF16); b2c = inp("b2c", (128, 1))
    lwWt = inp("lwWt", (8, 384)); lbp = inp("lbp", (8, 1))
    eye = inp("eye", (128, 128), BF16)
    out_f = nc.dram_tensor("out_f", [1, EF], F32, kind="ExternalOutput")

    with tile.TileContext(nc) as tc:
        with (
            tc.tile_pool(name="cst", bufs=1) as cp,
            tc.tile_pool(name="arena", bufs=1) as ar,
            tc.tile_pool(name="wk", bufs=2) as wp,
            tc.tile_pool(name="fin", bufs=1) as fp,
            tc.tile_pool(name="ps2", bufs=2, space="PSUM") as p2,
            tc.tile_pool(name="ps1", bufs=1, space="PSUM") as p1,
            tc.tile_pool(name="dram", bufs=1, space="DRAM") as dp,
        ):
            def ld(ap, shape, dt=F32, tag=None, rearr=None):
                t = cp.tile(list(shape), dt, tag=tag)
                src = ap[:] if rearr is None else ap[:].rearrange(rearr)
                nc.sync.dma_start(t[:], src)
                return t


            def lrelu_act(out, in_, bias):
                """out = rrelu(in_ + bias) = max(t, SLOPE*t), t = in_+bias."""
                P, Fr = out.shape[0], out.shape[-1]
                t = wp.tile([128, NT], F32, tag="lrt")
                st = wp.tile([128, NT], F32, tag="lrs")
                nc.scalar.activation(t[0:P, 0:Fr], in_, AF.Identity, bias=bias)
                nc.vector.tensor_scalar(st[0:P, 0:Fr], t[0:P, 0:Fr], SLOPE,
                                        None, op0=ALU.mult)
                nc.vector.tensor_tensor(out, t[0:P, 0:Fr], st[0:P, 0:Fr],
                                        op=ALU.max)

            c_Wln = ld(Wln, (8, 64), tag="cWln")
            c_bln = ld(bln, (64, 1), tag="cbln")
            c_leWt = ld(leWt, (19, 12), tag="cleWt")
            c_leb = ld(leb, (12, 1), tag="cleb")
            c_G1 = ld(G1, (128, 6, 64), BF16, "cG1")
            c_REP1 = ld(REP1, (12, 6, 128), BF16, "cREP1")
            c_G2 = ld(G2, (128, 8, 128), BF16, "cG2")
            c_REP2 = ld(REP2, (8, 8, 128), BF16, "cREP2")
            c_c1b = ld(c1b, (64, 1), tag="cc1b")
            c_c2b = ld(c2b, (128, 1), tag="cc2b")
            c_wih1 = ld(wih1, (64, 192), BF16, "cwih1")
            c_whh1 = ld(whh1, (64, 192), BF16, "cwhh1")
            c_br1 = ld(br1, (64, 1), tag="cbr1")
            c_bz1 = ld(bz1, (64, 1), tag="cbz1")
            c_bin1 = ld(bin1, (64, 1), tag="cbin1")
            c_bhn1 = ld(bhn1, (64, 1), tag="cbhn1")
            c_wih2 = ld(wih2, (128, 384), BF16, "cwih2")
            c_whh2 = ld(whh2, (128, 384), BF16, "cwhh2")
            c_br2 = ld(br2, (128, 1), tag="cbr2")
            c_bz2 = ld(bz2, (128, 1), tag="cbz2")
            c_bin2 = ld(bin2, (128, 1), tag="cbin2")
            c_bhn2 = ld(bhn2, (128, 1), tag="cbhn2")
            c_W1cT = ld(W1cT, (64, 128), BF16, "cW1cT")
            c_b1c = ld(b1c, (128, 1), tag="cb1c")
            c_W2cT = ld(W2cT, (128, 128), BF16, "cW2cT")
            c_b2c = ld(b2c, (128, 1), tag="cb2c")
            c_lwWt = ld(lwWt, (8, 384), tag="clwWt")
            c_lbp = ld(lbp, (8, 1), tag="clbp")
            c_eye = ld(eye, (128, 128), BF16, "ceye")
            c_inv1 = ld(invd1, (128, J1), tag="cinv1")
            c_inv3 = ld(invd3, (128, J3), tag="cinv3")
            c_gi1 = ld(gidx1, (128, EP1 // 16), I16, "cgi1")
            c_gi3 = ld(gidx3, (128, EP3 // 16), I16, "cgi3")
            c_gf0 = ld(gf0, (128, EF // 16), I16, "cgf0")
            c_gf1 = ld(gf1, (128, EF // 16), I16, "cgf1")
            c_ones = cp.tile([128, 1], F32, tag="cones")
            nc.vector.memset(c_ones[:], 1.0)

            # arenas (slot-shared across stages)
            A_eax = ar.tile([128, LE], BF16, tag="eax")
            A_z = ar.tile([128, LZ], BF16, tag="z")
            A_g = ar.tile([128, 1, LG], BF16, tag="g")
            A_me = ar.tile([128, LM], BF16, tag="me")
            A_hf = ar.tile([128, NL], F32, tag="hf")
            A_hb = ar.tile([128, NL], BF16, tag="hb")
            A_m = ar.tile([128, NL], BF16, tag="m")
            A_hn = ar.tile([128, NLP // 128, 128], BF16, tag="hx")

            H1_loc = dp.tile([NLP, 128], BF16)
            H1 = dp.tile([C * NLP, 128], BF16)
            H2_loc = dp.tile([NLP, 128], BF16)
            H2 = dp.tile([C * NLP, 128], BF16)

            def mov_chunks(n):
                return [(j, slice(j * NT, min((j + 1) * NT, n)),
                         min((j + 1) * NT, n) - j * NT)
                        for j in range((n + NT - 1) // NT)]

            # ---------------- encode
            sc = nc.named_scope
            with sc("encode"):
              for j, sl, w in mov_chunks(NL):
                xt = wp.tile([8, NT], F32, tag="xt")
                nc.sync.dma_start(xt[:, 0:w], xT[:, sl])
                p = p2.tile([64, NT], F32, tag="p512")
                nc.tensor.matmul(p[:, 0:w], c_Wln[:], xt[:, 0:w],
                                 start=True, stop=True)
                lrelu_act(A_hf[0:64, sl], p[:, 0:w], c_bln[:])
              nc.vector.tensor_copy(A_hb[0:64, :], A_hf[0:64, :])

            # ---------------- stage1 edge constants: ea1 + expanded chunks
            with sc("ea1"):
              for j, sl, w in mov_chunks(EP1):
                ea1t = wp.tile([19, NT], F32, tag="ea1t")
                nc.sync.dma_start(ea1t[:, 0:w], eaT1[:, sl])
                p = p2.tile([12, NT], F32, tag="p512")
                nc.tensor.matmul(p[:, 0:w], c_leWt[:], ea1t[:, 0:w],
                                 start=True, stop=True)
                ea1c = wp.tile([12, NT], BF16, tag="ea1c")
                lrelu_act(ea1c[:, 0:w], p[:, 0:w], c_leb[:])
                for cc in range(6):
                    pe = p2.tile([128, NT], F32, tag="p512")
                    nc.tensor.matmul(pe[:, 0:w], c_REP1[:, cc, :],
                                     ea1c[:, 0:w], start=True, stop=True)
                    nc.vector.tensor_copy(A_eax[:, cc * EP1 + sl.start:
                                                cc * EP1 + sl.stop], pe[:, 0:w])

            def export_state(feat, H_loc, H, hb, tag="exp"):
                """feature-major bf16 -> node-major rows + AllGather."""
                with sc(tag):
                  for j in range(NLP // 128):
                    lo, hi = j * 128, min((j + 1) * 128, NL)
                    w = hi - lo
                    src = wp.tile([feat, 128], BF16, tag="exs")
                    if w < 128:
                        nc.vector.memset(src[:], 0.0)
                    nc.vector.tensor_copy(src[0:feat, 0:w], hb[0:feat, lo:hi])
                    nc.sync.dma_start_transpose(A_hn[:, j, 0:feat],
                                                src[0:feat, :])
                    if feat == 64:
                        nc.sync.dma_start_transpose(A_hn[:, j, 64:128],
                                                    src[0:feat, :])
                  nc.sync.dma_start(
                      H_loc[:].rearrange("(j p) f -> p j f", p=128), A_hn[:])
                  nc.gpsimd.collective_compute(
                      "AllGather", ALU.bypass, replica_groups=[list(range(C))],
                      ins=[H_loc[:].opt()], outs=[H[:].opt()])

            def msg_pass(H, gi, EP, nk, Gc, inv, S_d, J, feat, m_out, cbias):
                GC = 256
                for gof in range(0, EP, GC):
                    gw = min(GC, EP - gof)
                    nc.gpsimd.dma_gather(
                        A_g[:, :, gof:gof + gw], H[:],
                        gi[:, gof // 16:(gof + gw) // 16], gw, gw, 128,
                        transpose=True)
                g2 = A_g[:].rearrange("p one e -> p (one e)")
                npass = 2 if feat == 64 else 1
                jb = [0] + ([(J // 2) * 128, EP] if npass == 2 else [EP])
                for ph in range(npass):
                    base, eph = jb[ph], jb[ph + 1] - jb[ph]
                    for kc in range(nk):
                        nc.vector.tensor_tensor(
                            A_z[:, kc * eph:(kc + 1) * eph],
                            g2[:, base:base + eph],
                            A_eax[:, kc * EP + base:kc * EP + base + eph],
                            op=ALU.mult)
                    for j, sl, w in mov_chunks(eph):
                        p = p2.tile([feat, NT], F32, tag="p512")
                        for kc in range(nk):
                            nc.tensor.matmul(
                                p[:, 0:w], Gc[:, kc, 0:feat],
                                A_z[:, kc * eph + sl.start:kc * eph + sl.stop],
                                start=(kc == 0), stop=(kc == nk - 1))
                        mc = wp.tile([feat, NT], BF16, tag="mc")
                        nc.vector.tensor_copy(mc[:, 0:w], p[:, 0:w])
                        for q in range(w // 128):
                            jj = (base + sl.start) // 128 + q
                            pt = p1.tile([128, feat], BF16, tag="ptr")
                            nc.tensor.transpose(
                                pt[:], mc[:, q * 128:(q + 1) * 128],
                                c_eye[0:feat, 0:feat])
                            nc.scalar.activation(
                                A_me[:, jj * feat:(jj + 1) * feat], pt[:],
                                AF.Identity, scale=inv[:, jj:jj + 1])
                cpt = J // NTC
                for t in range(NTC):
                    pm = p1.tile([feat, NT], F32, tag="pm")
                    for q in range(cpt):
                        j = t * cpt + q
                        sb = wp.tile([128, NT], BF16, tag="Sblk")
                        nc.sync.dma_start(sb[:], S_d[j])
                        nc.tensor.matmul(pm[:], A_me[:, j * feat:(j + 1) * feat],
                                         sb[:], start=(q == 0),
                                         stop=(q == cpt - 1))
                    hi = min(NT, NL - t * NT)
                    lrelu_act(m_out[0:feat, t * NT:t * NT + hi],
                              pm[:, 0:hi], cbias[:])

            def gru(dd, m_bf, wih, whh, bR, bZ, bI, bH):
                for t, sl, hi in mov_chunks(NL):
                    pR = p1.tile([dd, NT], F32, tag="pgR")
                    pZ = p1.tile([dd, NT], F32, tag="pgZ")
                    pI = p1.tile([dd, NT], F32, tag="pgI")
                    pH = p1.tile([dd, NT], F32, tag="pgH")
                    nc.tensor.matmul(pR[:, 0:hi], wih[:, 0:dd], m_bf[0:dd, sl],
                                     start=True, stop=False)
                    nc.tensor.matmul(pR[:, 0:hi], whh[:, 0:dd], A_hb[0:dd, sl],
                                     start=False, stop=True)
                    nc.tensor.matmul(pZ[:, 0:hi], wih[:, dd:2 * dd],
                                     m_bf[0:dd, sl], start=True, stop=False)
                    nc.tensor.matmul(pZ[:, 0:hi], whh[:, dd:2 * dd],
                                     A_hb[0:dd, sl], start=False, stop=True)
                    nc.tensor.matmul(pI[:, 0:hi], wih[:, 2 * dd:3 * dd],
                                     m_bf[0:dd, sl], start=True, stop=True)
                    nc.tensor.matmul(pH[:, 0:hi], whh[:, 2 * dd:3 * dd],
                                     A_hb[0:dd, sl], start=True, stop=True)
                    rs = wp.tile([dd, NT], F32, tag="grs")
                    zs = wp.tile([dd, NT], F32, tag="gzs")
                    nc.scalar.activation(rs[:, 0:hi], pR[:, 0:hi], AF.Sigmoid,
                                         bias=bR[:])
                    nc.scalar.activation(zs[:, 0:hi], pZ[:, 0:hi], AF.Sigmoid,
                                         bias=bZ[:])
                    hs = wp.tile([dd, NT], F32, tag="ghs")
                    nc.scalar.activation(hs[:, 0:hi], pH[:, 0:hi], AF.Identity,
                                         bias=bH[:])
                    t1 = wp.tile([dd, NT], F32, tag="gt1")
                    nc.vector.tensor_tensor(t1[:, 0:hi], rs[:, 0:hi],
                                            hs[:, 0:hi], op=ALU.mult)
                    nc.vector.tensor_tensor(t1[:, 0:hi], t1[:, 0:hi],
                                            pI[:, 0:hi], op=ALU.add)
                    nt_ = wp.tile([dd, NT], F32, tag="gnt")
                    nc.scalar.activation(nt_[:, 0:hi], t1[:, 0:hi], AF.Tanh,
                                         bias=bI[:])
                    hm = wp.tile([dd, NT], F32, tag="ghm")
                    nc.vector.tensor_tensor(hm[:, 0:hi], A_hf[0:dd, sl],
                                            nt_[:, 0:hi], op=ALU.subtract)
                    nc.vector.tensor_tensor(hm[:, 0:hi], hm[:, 0:hi],
                                            zs[:, 0:hi], op=ALU.mult)
                    nc.vector.tensor_tensor(A_hf[0:dd, sl], hm[:, 0:hi],
                                            nt_[:, 0:hi], op=ALU.add)
                nc.vector.tensor_copy(A_hb[0:dd, :], A_hf[0:dd, :])

            # ---------------- stage 1
            export_state(64, H1_loc, H1, A_hb)
            for it in range(2):
                msg_pass(H1, c_gi1, EP1, 6, c_G1, c_inv1, S1, J1, 64,
                         A_m, c_c1b)
                gru(64, A_m, c_wih1, c_whh1, c_br1, c_bz1, c_bin1, c_bhn1)
                if it == 0:
                    export_state(64, H1_loc, H1, A_hb)

            # ---------------- lin_covert (h: 64 -> 128 features)
            for j, sl, w in mov_chunks(NL):
                p = p2.tile([128, NT], F32, tag="p512")
                nc.tensor.matmul(p[:, 0:w], c_W1cT[:], A_hb[0:64, sl],
                                 start=True, stop=True)
                lrelu_act(A_m[:, sl], p[:, 0:w], c_b1c[:])
            for j, sl, w in mov_chunks(NL):
                p = p2.tile([128, NT], F32, tag="p512")
                nc.tensor.matmul(p[:, 0:w], c_W2cT[:], A_m[:, sl],
                                 start=True, stop=True)
                lrelu_act(A_hf[:, sl], p[:, 0:w], c_b2c[:])
            nc.vector.tensor_copy(A_hb[:], A_hf[:])

            # ---------------- stage2 edge constants (reuse eax arena)
            for j, sl, w in mov_chunks(EP3):
                ea3f = wp.tile([8, NT], F32, tag="ea3f")
                nc.sync.dma_start(ea3f[:, 0:w], eaT3[:, sl])
                ea3b = wp.tile([8, NT], BF16, tag="ea3b")
                nc.vector.tensor_copy(ea3b[:, 0:w], ea3f[:, 0:w])
                for k in range(8):
                    p = p2.tile([128, NT], F32, tag="p512")
                    nc.tensor.matmul(p[:, 0:w], c_REP2[:, k, :], ea3b[:, 0:w],
                                     start=True, stop=True)
                    nc.vector.tensor_copy(A_eax[:, k * EP3 + sl.start:
                                                k * EP3 + sl.stop], p[:, 0:w])

            # ---------------- stage 2
            export_state(128, H2_loc, H2, A_hb)
            for it in range(2):
                msg_pass(H2, c_gi3, EP3, 8, c_G2, c_inv3, S3, J3, 128,
                         A_m, c_c2b)
                gru(128, A_m, c_wih2, c_whh2, c_br2, c_bz2, c_bin2, c_bhn2)
                export_state(128, H2_loc, H2, A_hb)

            # ---------------- final readout
            t0 = fp.tile([128, 1, EF], BF16, tag="t0")
            t1_ = fp.tile([128, 1, EF], BF16, tag="t1")
            for gof in range(0, EF, 256):
                nc.gpsimd.dma_gather(
                    t0[:, :, gof:gof + 256], H2[:],
                    c_gf0[:, gof // 16:(gof + 256) // 16], 256, 256, 128,
                    transpose=True)
                nc.gpsimd.dma_gather(
                    t1_[:, :, gof:gof + 256], H2[:],
                    c_gf1[:, gof // 16:(gof + 256) // 16], 256, 256, 128,
                    transpose=True)
            a0 = t0[:].rearrange("p one e -> p (one e)")
            a1 = t1_[:].rearrange("p one e -> p (one e)")
            ea3l = fp.tile([8, EF], F32, tag="ea3l")
            nc.sync.dma_start(ea3l[:], ea3locT[:])
            fB0 = fp.tile([128, EF], F32, tag="fB0")
            fB1 = fp.tile([128, EF], F32, tag="fB1")
            fB2 = fp.tile([128, EF], F32, tag="fB2")
            fB = [fB0, fB1, fB2]
            nc.vector.tensor_tensor(fB[0][:], a0, a1, op=ALU.add)
            nc.vector.tensor_tensor(fB[1][:], a0, a1, op=ALU.mult)
            nc.vector.tensor_tensor(fB[2][:], a0, a1, op=ALU.subtract)
            nc.vector.tensor_tensor(fB[2][:], fB[2][:], fB[2][:], op=ALU.mult)
            pacc = p1.tile([1, EF], F32, tag="pm")
            for bi in range(3):
                pw = p2.tile([128, EF], F32, tag="p512")
                nc.tensor.matmul(pw[:], c_lwWt[:, bi * 128:(bi + 1) * 128],
                                 ea3l[:], start=True, stop=True)
                pr = fp.tile([128, EF], F32, tag="prod")
                nc.vector.tensor_tensor(pr[:], fB[bi][:], pw[:], op=ALU.mult)
                nc.tensor.matmul(pacc[:], c_ones[:], pr[:],
                                 start=(bi == 0), stop=False)
            nc.tensor.matmul(pacc[:], c_lbp[:], ea3l[:], start=False, stop=True)
            ot = fp.tile([1, EF], F32, tag="ot")
            nc.vector.tensor_copy(ot[:], pacc[:])
            nc.sync.dma_start(out_f[:], ot[:])

    nc.compile()
    return nc


_CACHE = {}


def kernel(**inputs):
    static, in_maps = _host_prep(inputs)
    if static not in _CACHE:
        _CACHE[static] = _build(*static)
    nc = _CACHE[static]
    res = run_bass_kernel_spmd(nc, in_maps, list(range(C))).results
    return np.concatenate(
        [res[c]["out_f"][0, :E3 // C] for c in range(C)]).astype(np.float32)



# revision 16
# speedup vs baseline: 471.4620x; 471.4620x over previous
"""GNN message-passing kernel for 8 Trainium2 NeuronCores (Bass/Tile).

Sharding: each core owns 2500 nodes + all edges targeting them. Node state
is feature-major in SBUF; after each GRU update it is written node-major
(bf16, 128-wide rows) to DRAM and AllGathered so any core can dma_gather
arbitrary source rows.  Per-edge weights are never materialized:
  msg_e = h[src_e] @ (ea_e @ nnW^T).reshape(D,D)
is computed as  Z[(k,i),e] = ea[k,e] * h[i,src_e];  msg = G^T @ Z
with G a host-prepacked rearrangement of nnW.  segment-sum over targets is
a matmul against host-built 0/1 staircase blocks (edges sorted by target,
each 512-node tile padded to an integral number of 128-edge chunks); 1/deg
is applied per-edge (exact fp32) on the scalar engine during PSUM evac.
"""
import sys
sys.path.insert(0, "/opt/trn_rl_repo")
import numpy as np
import ml_dtypes

import concourse.bass as bass
import concourse.bacc as bacc
import concourse.mybir as mybir
import concourse.tile as tile
from concourse.bass_utils import run_bass_kernel_spmd

F32 = mybir.dt.float32
BF16 = mybir.dt.bfloat16
I16 = mybir.dt.int16
AF = mybir.ActivationFunctionType
ALU = mybir.AluOpType

N, E, E3, D = 20000, 30000, 4000, 64
D2 = 2 * D
C = 8
NL = N // C          # nodes per core (2500)
NLP = 2560           # padded rows per core in gathered state (mult of 128)
NT = 512             # node-tile / matmul moving chunk
NTC = (NL + NT - 1) // NT
SLOPE = (1.0 / 8.0 + 1.0 / 3.0) / 2.0
EPS = 1e-5
EF = 512             # final readout edges per core (500 real)

bfd = ml_dtypes.bfloat16


# ----------------------------------------------------------------- host prep

def _wrap16(idx):
    n = len(idx)
    w = idx.reshape(n // 16, 16).T.astype(np.int16)
    return np.tile(w, (8, 1)).copy()


def _pad_id(n):
    """global node id -> padded row id in gathered state."""
    return (n // NL) * NLP + (n % NL)


def _affine_bn(g, be, m, v):
    a = g / np.sqrt(v + EPS)
    return a, be - m * a


def _prep_edges(src, tgt, attr, n_attr):
    owner = tgt // NL
    per_core = []
    maxrun = 1
    for c in range(C):
        sel = np.where(owner == c)[0]
        tl = tgt[sel] - c * NL
        order = np.argsort(tl, kind="stable")
        sel, tl = sel[order], tl[order]
        per_core.append((sel, tl))
        for t in range(NTC):
            maxrun = max(maxrun, int(((tl // NT) == t).sum()))
    cpt = (maxrun + 127) // 128
    ep = NTC * cpt * 128

    gidx = np.zeros((C, ep), np.int64)
    eaT = np.zeros((C, n_attr, ep), np.float32)
    invdeg_e = np.zeros((C, ep), np.float32)
    s_blocks = np.zeros((C, NTC * cpt, 128, NT), bfd)
    deg = np.maximum(np.bincount(tgt, minlength=N), 1).astype(np.float32)

    for c, (sel, tl) in enumerate(per_core):
        for t in range(NTC):
            msk = (tl // NT) == t
            idxs, tls = sel[msk], tl[msk]
            k = len(idxs)
            pos = t * cpt * 128
            gidx[c, pos:pos + k] = _pad_id(src[idxs])
            eaT[c, :, pos:pos + k] = attr[idxs].T
            invdeg_e[c, pos:pos + k] = 1.0 / deg[c * NL + tls]
            rel = tls - t * NT
            ar = np.arange(k) + pos
            s_blocks[c, ar // 128, ar % 128, rel] = 1.0
    return dict(ep=ep, gidx=gidx, eaT=eaT, invdeg_e=invdeg_e, s=s_blocks)


def _host_prep(inp):
    g = lambda k: np.asarray(inp[k], np.float32)
    ei = np.asarray(inp["edge_index"], np.int64)
    ei3 = np.asarray(inp["edge_index3"], np.int64)

    a, b = _affine_bn(g("nx_g"), g("nx_be"), g("nx_m"), g("nx_v"))
    Wln = (a[:, None] * g("ln_W").T).astype(np.float32)
    bln = (b @ g("ln_W").T + g("ln_b")).astype(np.float32)

    e1 = _prep_edges(ei[0], ei[1], g("edge_attr"), 19)
    nn1 = g("nn1_W")
    G1 = np.zeros((128, 6, 64), np.float32)           # partition-first
    REP1 = np.zeros((12, 6, 128), np.float32)
    for cc in range(6):
        for half, k in enumerate((2 * cc, 2 * cc + 1)):
            G1[half * 64:(half + 1) * 64, cc, :] = nn1[:, k].reshape(64, 64)
            REP1[k, cc, half * 64:(half + 1) * 64] = 1.0

    src3 = np.concatenate([ei3[0], ei3[1]])
    tgt3 = np.concatenate([ei3[1], ei3[0]])
    attr3 = np.concatenate([g("edge_attr3"), g("edge_attr3")], axis=0)
    e2 = _prep_edges(src3, tgt3, attr3, 8)
    nn2 = g("nn2_W")
    G2 = np.zeros((128, 8, 128), np.float32)
    REP2 = np.zeros((8, 8, 128), np.float32)
    for k in range(8):
        G2[:, k, :] = nn2[:, k].reshape(D2, D2)
        REP2[k, k, :] = 1.0

    f_i0 = np.zeros((C, EF), np.int64)
    f_i1 = np.zeros((C, EF), np.int64)
    ea3locT = np.zeros((C, 8, EF), np.float32)
    npc = E3 // C
    for c in range(C):
        lo = c * npc
        f_i0[c, :npc] = _pad_id(ei3[0, lo:lo + npc])
        f_i1[c, :npc] = _pad_id(ei3[1, lo:lo + npc])
        ea3locT[c, :, :npc] = g("edge_attr3")[lo:lo + npc].T

    a_nm, b_nm = _affine_bn(g("nm_g"), g("nm_be"), g("nm_m"), g("nm_v"))
    a_nm = a_nm.copy()
    a_nm[0:D2] *= 0.5
    lwWt = (g("lw_W") * a_nm[:, None]).T.astype(np.float32)   # (8,384)
    lbp = (g("lb_W")[0] + b_nm @ g("lw_W")).astype(np.float32)

    alc, blc = _affine_bn(g("lc_g"), g("lc_be"), g("lc_m"), g("lc_v"))
    W1c = g("lc_w1") * alc[None, :]
    b1c = (g("lc_w1") @ blc + g("lc_b1")).astype(np.float32)

    bih1, bhh1 = g("g1_bih"), g("g1_bhh")
    bih2, bhh2 = g("g2_bih"), g("g2_bhh")

    xs = g("x")
    in_maps = []
    for c in range(C):
        m = {
            "xT": xs[c * NL:(c + 1) * NL].T,
            "eaT1": e1["eaT"][c],
            "gidx1": _wrap16(e1["gidx"][c]),
            "invd1": e1["invdeg_e"][c].reshape(-1, 128).T,
            "S1": e1["s"][c],
            "eaT3": e2["eaT"][c],
            "gidx3": _wrap16(e2["gidx"][c]),
            "invd3": e2["invdeg_e"][c].reshape(-1, 128).T,
            "S3": e2["s"][c],
            "gf0": _wrap16(f_i0[c]), "gf1": _wrap16(f_i1[c]),
            "ea3locT": ea3locT[c],
            "Wln": Wln, "bln": bln.reshape(-1, 1),
            "leWt": g("le_W").T, "leb": g("le_b").reshape(-1, 1),
            "G1": G1.astype(bfd), "REP1": REP1.astype(bfd),
            "G2": G2.astype(bfd), "REP2": REP2.astype(bfd),
            "c1b": g("c1_b").reshape(-1, 1), "c2b": g("c2_b").reshape(-1, 1),
            "wih1": g("g1_wih").T.astype(bfd), "whh1": g("g1_whh").T.astype(bfd),
            "br1": (bih1 + bhh1)[0:D].reshape(-1, 1),
            "bz1": (bih1 + bhh1)[D:2 * D].reshape(-1, 1),
            "bin1": bih1[2 * D:].reshape(-1, 1),
            "bhn1": bhh1[2 * D:].reshape(-1, 1),
            "wih2": g("g2_wih").T.astype(bfd), "whh2": g("g2_whh").T.astype(bfd),
            "br2": (bih2 + bhh2)[0:D2].reshape(-1, 1),
            "bz2": (bih2 + bhh2)[D2:2 * D2].reshape(-1, 1),
            "bin2": bih2[2 * D2:].reshape(-1, 1),
            "bhn2": bhh2[2 * D2:].reshape(-1, 1),
            "W1cT": W1c.T.astype(bfd), "b1c": b1c.reshape(-1, 1),
            "W2cT": g("lc_w2").T.astype(bfd), "b2c": g("lc_b2").reshape(-1, 1),
            "lwWt": lwWt, "lbp": lbp.reshape(-1, 1),
            "eye": np.eye(128, dtype=bfd),
        }
        in_maps.append({k: np.ascontiguousarray(v) for k, v in m.items()})
    static = (e1["ep"], e2["ep"])
    return static, in_maps


# ------------------------------------------------------------- kernel builder

def _build(EP1, EP3):
    nc = bacc.Bacc("TRN2", target_bir_lowering=False, debug=False,
                   num_devices=C)
    J1, J3 = EP1 // 128, EP3 // 128
    LZ = max(6 * ((EP1 // 128 + 1) // 2 + 1) * 128, 8 * EP3)  # z arena
    LE = max(6 * EP1, 8 * EP3)            # eax arena
    LG = max(EP1, EP3)
    LM = max(J1 * 64, J3 * 128)           # msg_em arena

    def inp(name, shape, dt=F32):
        return nc.dram_tensor(name, list(shape), dt, kind="ExternalInput")

    xT = inp("xT", (8, NL))
    eaT1 = inp("eaT1", (19, EP1)); gidx1 = inp("gidx1", (128, EP1 // 16), I16)
    invd1 = inp("invd1", (128, J1)); S1 = inp("S1", (J1, 128, NT), BF16)
    eaT3 = inp("eaT3", (8, EP3)); gidx3 = inp("gidx3", (128, EP3 // 16), I16)
    invd3 = inp("invd3", (128, J3)); S3 = inp("S3", (J3, 128, NT), BF16)
    gf0 = inp("gf0", (128, EF // 16), I16); gf1 = inp("gf1", (128, EF // 16), I16)
    ea3locT = inp("ea3locT", (8, EF))
    Wln = inp("Wln", (8, 64)); bln = inp("bln", (64, 1))
    leWt = inp("leWt", (19, 12)); leb = inp("leb", (12, 1))
    G1 = inp("G1", (128, 6, 64), BF16); REP1 = inp("REP1", (12, 6, 128), BF16)
    G2 = inp("G2", (128, 8, 128), BF16); REP2 = inp("REP2", (8, 8, 128), BF16)
    c1b = inp("c1b", (64, 1)); c2b = inp("c2b", (128, 1))
    wih1 = inp("wih1", (64, 192), BF16); whh1 = inp("whh1", (64, 192), BF16)
    br1 = inp("br1", (64, 1)); bz1 = inp("bz1", (64, 1))
    bin1 = inp("bin1", (64, 1)); bhn1 = inp("bhn1", (64, 1))
    wih2 = inp("wih2", (128, 384), BF16); whh2 = inp("whh2", (128, 384), BF16)
    br2 = inp("br2", (128, 1)); bz2 = inp("bz2", (128, 1))
    bin2 = inp("bin2", (128, 1)); bhn2 = inp("bhn2", (128, 1))
    W1cT = inp("W1cT", (64, 128), BF16); b1c = inp("b1c", (128, 1))
    W2cT = inp("W2cT", (128, 128), BF16); b2c = inp("b2c", (128, 1))
    lwWt = inp("lwWt", (8, 384)); lbp = inp("lbp", (8, 1))
    eye = inp("eye", (128, 128), BF16)
    out_f = nc.dram_tensor("out_f", [1, EF], F32, kind="ExternalOutput")

    with tile.TileContext(nc) as tc:
        with (
            tc.tile_pool(name="cst", bufs=1) as cp,
            tc.tile_pool(name="arena", bufs=1) as ar,
            tc.tile_pool(name="wk", bufs=2) as wp,
            tc.tile_pool(name="fin", bufs=1) as fp,
            tc.tile_pool(name="ps2", bufs=2, space="PSUM") as p2,
            tc.tile_pool(name="ps1", bufs=1, space="PSUM") as p1,
            tc.tile_pool(name="dram", bufs=1, space="DRAM") as dp,
        ):
            def ld(ap, shape, dt=F32, tag=None, rearr=None):
                t = cp.tile(list(shape), dt, tag=tag)
                src = ap[:] if rearr is None else ap[:].rearrange(rearr)
                nc.sync.dma_start(t[:], src)
                return t


            def lrelu_act(out, in_, bias):
                """out = rrelu(in_ + bias) = max(t, SLOPE*t), t = in_+bias."""
                P, Fr = out.shape[0], out.shape[-1]
                t = wp.tile([128, NT], F32, tag="lrt")
                st = wp.tile([128, NT], F32, tag="lrs")
                nc.scalar.activation(t[0:P, 0:Fr], in_, AF.Identity, bias=bias)
                nc.vector.tensor_scalar(st[0:P, 0:Fr], t[0:P, 0:Fr], SLOPE,
                                        None, op0=ALU.mult)
                nc.vector.tensor_tensor(out, t[0:P, 0:Fr], st[0:P, 0:Fr],
                                        op=ALU.max)

            c_Wln = ld(Wln, (8, 64), tag="cWln")
            c_bln = ld(bln, (64, 1), tag="cbln")
            c_leWt = ld(leWt, (19, 12), tag="cleWt")
            c_leb = ld(leb, (12, 1), tag="cleb")
            c_G1 = ld(G1, (128, 6, 64), BF16, "cG1")
            c_REP1 = ld(REP1, (12, 6, 128), BF16, "cREP1")
            c_G2 = ld(G2, (128, 8, 128), BF16, "cG2")
            c_REP2 = ld(REP2, (8, 8, 128), BF16, "cREP2")
            c_c1b = ld(c1b, (64, 1), tag="cc1b")
            c_c2b = ld(c2b, (128, 1), tag="cc2b")
            c_wih1 = ld(wih1, (64, 192), BF16, "cwih1")
            c_whh1 = ld(whh1, (64, 192), BF16, "cwhh1")
            c_br1 = ld(br1, (64, 1), tag="cbr1")
            c_bz1 = ld(bz1, (64, 1), tag="cbz1")
            c_bin1 = ld(bin1, (64, 1), tag="cbin1")
            c_bhn1 = ld(bhn1, (64, 1), tag="cbhn1")
            c_wih2 = ld(wih2, (128, 384), BF16, "cwih2")
            c_whh2 = ld(whh2, (128, 384), BF16, "cwhh2")
            c_br2 = ld(br2, (128, 1), tag="cbr2")
            c_bz2 = ld(bz2, (128, 1), tag="cbz2")
            c_bin2 = ld(bin2, (128, 1), tag="cbin2")
            c_bhn2 = ld(bhn2, (128, 1), tag="cbhn2")
            c_W1cT = ld(W1cT, (64, 128), BF16, "cW1cT")
            c_b1c = ld(b1c, (128, 1), tag="cb1c")
            c_W2cT = ld(W2cT, (128, 128), BF16, "cW2cT")
            c_b2c = ld(b2c, (128, 1), tag="cb2c")
            c_lwWt = ld(lwWt, (8, 384), tag="clwWt")
            c_lbp = ld(lbp, (8, 1), tag="clbp")
            c_eye = ld(eye, (128, 128), BF16, "ceye")
            c_inv1 = ld(invd1, (128, J1), tag="cinv1")
            c_inv3 = ld(invd3, (128, J3), tag="cinv3")
            c_gi1 = ld(gidx1, (128, EP1 // 16), I16, "cgi1")
            c_gi3 = ld(gidx3, (128, EP3 // 16), I16, "cgi3")
            c_gf0 = ld(gf0, (128, EF // 16), I16, "cgf0")
            c_gf1 = ld(gf1, (128, EF // 16), I16, "cgf1")
            c_ones = cp.tile([128, 1], F32, tag="cones")
            nc.vector.memset(c_ones[:], 1.0)

            # arenas (slot-shared across stages)
            A_eax = ar.tile([128, LE], BF16, tag="eax")
            A_z = ar.tile([128, LZ], BF16, tag="z")
            A_g = ar.tile([128, 1, LG], BF16, tag="g")
            A_me = ar.tile([128, LM], BF16, tag="me")
            A_hf = ar.tile([128, NL], F32, tag="hf")
            A_hb = ar.tile([128, NL], BF16, tag="hb")
            A_m = ar.tile([128, NL], BF16, tag="m")
            A_hn = ar.tile([128, NLP // 128, 128], BF16, tag="hx")

            H1_loc = dp.tile([NLP, 128], BF16)
            H1a = dp.tile([C * NLP, 128], BF16, addr_space="Shared")
            H1b = dp.tile([C * NLP, 128], BF16, addr_space="Shared")
            H2_loc = dp.tile([NLP, 128], BF16)
            H2a = dp.tile([C * NLP, 128], BF16, addr_space="Shared")
            H2b = dp.tile([C * NLP, 128], BF16, addr_space="Shared")
            H2c = dp.tile([C * NLP, 128], BF16, addr_space="Shared")

            def mov_chunks(n):
                return [(j, slice(j * NT, min((j + 1) * NT, n)),
                         min((j + 1) * NT, n) - j * NT)
                        for j in range((n + NT - 1) // NT)]

            # ---------------- encode
            sc = nc.named_scope
            with sc("encode"):
              for j, sl, w in mov_chunks(NL):
                xt = wp.tile([8, NT], F32, tag="xt")
                nc.sync.dma_start(xt[:, 0:w], xT[:, sl])
                p = p2.tile([64, NT], F32, tag="p512")
                nc.tensor.matmul(p[:, 0:w], c_Wln[:], xt[:, 0:w],
                                 start=True, stop=True)
                lrelu_act(A_hf[0:64, sl], p[:, 0:w], c_bln[:])
              nc.vector.tensor_copy(A_hb[0:64, :], A_hf[0:64, :])

            # ---------------- stage1 edge constants: ea1 + expanded chunks
            with sc("ea1"):
              for j, sl, w in mov_chunks(EP1):
                ea1t = wp.tile([19, NT], F32, tag="ea1t")
                nc.sync.dma_start(ea1t[:, 0:w], eaT1[:, sl])
                p = p2.tile([12, NT], F32, tag="p512")
                nc.tensor.matmul(p[:, 0:w], c_leWt[:], ea1t[:, 0:w],
                                 start=True, stop=True)
                ea1c = wp.tile([12, NT], BF16, tag="ea1c")
                lrelu_act(ea1c[:, 0:w], p[:, 0:w], c_leb[:])
                for cc in range(6):
                    pe = p2.tile([128, NT], F32, tag="p512")
                    nc.tensor.matmul(pe[:, 0:w], c_REP1[:, cc, :],
                                     ea1c[:, 0:w], start=True, stop=True)
                    nc.vector.tensor_copy(A_eax[:, cc * EP1 + sl.start:
                                                cc * EP1 + sl.stop], pe[:, 0:w])

            def export_state(feat, H_loc, H, hb, tag="exp"):
                """feature-major bf16 -> node-major rows + AllGather."""
                with sc(tag):
                  for j in range(NLP // 128):
                    lo, hi = j * 128, min((j + 1) * 128, NL)
                    w = hi - lo
                    src = wp.tile([feat, 128], BF16, tag="exs")
                    if w < 128:
                        nc.vector.memset(src[:], 0.0)
                    nc.vector.tensor_copy(src[0:feat, 0:w], hb[0:feat, lo:hi])
                    nc.sync.dma_start_transpose(A_hn[:, j, 0:feat],
                                                src[0:feat, :])
                    if feat == 64:
                        nc.sync.dma_start_transpose(A_hn[:, j, 64:128],
                                                    src[0:feat, :])
                  nc.sync.dma_start(
                      H_loc[:].rearrange("(j p) f -> p j f", p=128), A_hn[:])
                  nc.gpsimd.collective_compute(
                      "AllGather", ALU.bypass, replica_groups=[list(range(C))],
                      ins=[H_loc[:].opt()], outs=[H[:].opt()])

            def msg_pass(H, gi, EP, nk, Gc, inv, S_d, J, feat, m_out, cbias,
                         tag="mp"):
              with sc(tag):
                GC = 256
                for gof in range(0, EP, GC):
                    gw = min(GC, EP - gof)
                    nc.gpsimd.dma_gather(
                        A_g[:, :, gof:gof + gw], H[:],
                        gi[:, gof // 16:(gof + gw) // 16], gw, gw, 128,
                        transpose=True)
                g2 = A_g[:].rearrange("p one e -> p (one e)")
                npass = 2 if feat == 64 else 1
                jb = [0] + ([(J // 2) * 128, EP] if npass == 2 else [EP])
                for ph in range(npass):
                    base, eph = jb[ph], jb[ph + 1] - jb[ph]
                    for kc in range(nk):
                        nc.vector.tensor_tensor(
                            A_z[:, kc * eph:(kc + 1) * eph],
                            g2[:, base:base + eph],
                            A_eax[:, kc * EP + base:kc * EP + base + eph],
                            op=ALU.mult)
                    for j, sl, w in mov_chunks(eph):
                        p = p2.tile([feat, NT], F32, tag="p512")
                        for kc in range(nk):
                            nc.tensor.matmul(
                                p[:, 0:w], Gc[:, kc, 0:feat],
                                A_z[:, kc * eph + sl.start:kc * eph + sl.stop],
                                start=(kc == 0), stop=(kc == nk - 1))
                        mc = wp.tile([feat, NT], BF16, tag="mc")
                        nc.vector.tensor_copy(mc[:, 0:w], p[:, 0:w])
                        for q in range(w // 128):
                            jj = (base + sl.start) // 128 + q
                            pt = p1.tile([128, feat], BF16, tag="ptr")
                            nc.tensor.transpose(
                                pt[:], mc[:, q * 128:(q + 1) * 128],
                                c_eye[0:feat, 0:feat])
                            nc.scalar.activation(
                                A_me[:, jj * feat:(jj + 1) * feat], pt[:],
                                AF.Identity, scale=inv[:, jj:jj + 1])
                cpt = J // NTC
                for t in range(NTC):
                    pm = p1.tile([feat, NT], F32, tag="pm")
                    for q in range(cpt):
                        j = t * cpt + q
                        sb = wp.tile([128, NT], BF16, tag="Sblk")
                        nc.sync.dma_start(sb[:], S_d[j])
                        nc.tensor.matmul(pm[:], A_me[:, j * feat:(j + 1) * feat],
                                         sb[:], start=(q == 0),
                                         stop=(q == cpt - 1))
                    hi = min(NT, NL - t * NT)
                    lrelu_act(m_out[0:feat, t * NT:t * NT + hi],
                              pm[:, 0:hi], cbias[:])

            def gru(dd, m_bf, wih, whh, bR, bZ, bI, bH, tag="gru"):
              with sc(tag):
                for t, sl, hi in mov_chunks(NL):
                    pR = p1.tile([dd, NT], F32, tag="pgR")
                    pZ = p1.tile([dd, NT], F32, tag="pgZ")
                    pI = p1.tile([dd, NT], F32, tag="pgI")
                    pH = p1.tile([dd, NT], F32, tag="pgH")
                    nc.tensor.matmul(pR[:, 0:hi], wih[:, 0:dd], m_bf[0:dd, sl],
                                     start=True, stop=False)
                    nc.tensor.matmul(pR[:, 0:hi], whh[:, 0:dd], A_hb[0:dd, sl],
                                     start=False, stop=True)
                    nc.tensor.matmul(pZ[:, 0:hi], wih[:, dd:2 * dd],
                                     m_bf[0:dd, sl], start=True, stop=False)
                    nc.tensor.matmul(pZ[:, 0:hi], whh[:, dd:2 * dd],
                                     A_hb[0:dd, sl], start=False, stop=True)
                    nc.tensor.matmul(pI[:, 0:hi], wih[:, 2 * dd:3 * dd],
                                     m_bf[0:dd, sl], start=True, stop=True)
                    nc.tensor.matmul(pH[:, 0:hi], whh[:, 2 * dd:3 * dd],
                                     A_hb[0:dd, sl], start=True, stop=True)
                    rs = wp.tile([dd, NT], F32, tag="grs")
                    zs = wp.tile([dd, NT], F32, tag="gzs")
                    nc.scalar.activation(rs[:, 0:hi], pR[:, 0:hi], AF.Sigmoid,
                                         bias=bR[:])
                    nc.scalar.activation(zs[:, 0:hi], pZ[:, 0:hi], AF.Sigmoid,
                                         bias=bZ[:])
                    hs = wp.tile([dd, NT], F32, tag="ghs")
                    nc.scalar.activation(hs[:, 0:hi], pH[:, 0:hi], AF.Identity,
                                         bias=bH[:])
                    t1 = wp.tile([dd, NT], F32, tag="gt1")
                    nc.vector.tensor_tensor(t1[:, 0:hi], rs[:, 0:hi],
                                            hs[:, 0:hi], op=ALU.mult)
                    nc.vector.tensor_tensor(t1[:, 0:hi], t1[:, 0:hi],
                                            pI[:, 0:hi], op=ALU.add)
                    nt_ = wp.tile([dd, NT], F32, tag="gnt")
                    nc.scalar.activation(nt_[:, 0:hi], t1[:, 0:hi], AF.Tanh,
                                         bias=bI[:])
                    hm = wp.tile([dd, NT], F32, tag="ghm")
                    nc.vector.tensor_tensor(hm[:, 0:hi], A_hf[0:dd, sl],
                                            nt_[:, 0:hi], op=ALU.subtract)
                    nc.vector.tensor_tensor(hm[:, 0:hi], hm[:, 0:hi],
                                            zs[:, 0:hi], op=ALU.mult)
                    nc.vector.tensor_tensor(A_hf[0:dd, sl], hm[:, 0:hi],
                                            nt_[:, 0:hi], op=ALU.add)
                nc.vector.tensor_copy(A_hb[0:dd, :], A_hf[0:dd, :])

            # ---------------- stage 1
            H1s = [H1a, H1b]
            export_state(64, H1_loc, H1s[0], A_hb, tag="exp1_0")
            for it in range(2):
                msg_pass(H1s[it], c_gi1, EP1, 6, c_G1, c_inv1, S1, J1, 64,
                         A_m, c_c1b, tag=f"mp1_{it}")
                gru(64, A_m, c_wih1, c_whh1, c_br1, c_bz1, c_bin1, c_bhn1,
                    tag=f"gru1_{it}")
                if it == 0:
                    export_state(64, H1_loc, H1s[1], A_hb, tag="exp1_1")

            # ---------------- lin_covert (h: 64 -> 128 features)
            with sc("covert"):
              for j, sl, w in mov_chunks(NL):
                p = p2.tile([128, NT], F32, tag="p512")
                nc.tensor.matmul(p[:, 0:w], c_W1cT[:], A_hb[0:64, sl],
                                 start=True, stop=True)
                lrelu_act(A_m[:, sl], p[:, 0:w], c_b1c[:])
              for j, sl, w in mov_chunks(NL):
                p = p2.tile([128, NT], F32, tag="p512")
                nc.tensor.matmul(p[:, 0:w], c_W2cT[:], A_m[:, sl],
                                 start=True, stop=True)
                lrelu_act(A_hf[:, sl], p[:, 0:w], c_b2c[:])
              nc.vector.tensor_copy(A_hb[:], A_hf[:])

            # ---------------- stage2 edge constants (reuse eax arena)
            with sc("ea3"):
              for j, sl, w in mov_chunks(EP3):
                ea3f = wp.tile([8, NT], F32, tag="ea3f")
                nc.sync.dma_start(ea3f[:, 0:w], eaT3[:, sl])
                ea3b = wp.tile([8, NT], BF16, tag="ea3b")
                nc.vector.tensor_copy(ea3b[:, 0:w], ea3f[:, 0:w])
                for k in range(8):
                    p = p2.tile([128, NT], F32, tag="p512")
                    nc.tensor.matmul(p[:, 0:w], c_REP2[:, k, :], ea3b[:, 0:w],
                                     start=True, stop=True)
                    nc.vector.tensor_copy(A_eax[:, k * EP3 + sl.start:
                                                k * EP3 + sl.stop], p[:, 0:w])

            # ---------------- stage 2
            H2s = [H2a, H2b, H2c]
            export_state(128, H2_loc, H2s[0], A_hb, tag="exp2_0")
            for it in range(2):
                msg_pass(H2s[it], c_gi3, EP3, 8, c_G2, c_inv3, S3, J3, 128,
                         A_m, c_c2b, tag=f"mp2_{it}")
                gru(128, A_m, c_wih2, c_whh2, c_br2, c_bz2, c_bin2, c_bhn2,
                    tag=f"gru2_{it}")
                export_state(128, H2_loc, H2s[it + 1], A_hb,
                             tag=f"exp2_{it + 1}")

            # ---------------- final readout
            with sc("final"):
              t0 = fp.tile([128, 1, EF], BF16, tag="t0")
              t1_ = fp.tile([128, 1, EF], BF16, tag="t1")
              for gof in range(0, EF, 256):
                nc.gpsimd.dma_gather(
                    t0[:, :, gof:gof + 256], H2c[:],
                    c_gf0[:, gof // 16:(gof + 256) // 16], 256, 256, 128,
                    transpose=True)
                nc.gpsimd.dma_gather(
                    t1_[:, :, gof:gof + 256], H2c[:],
                    c_gf1[:, gof // 16:(gof + 256) // 16], 256, 256, 128,
                    transpose=True)
              a0 = t0[:].rearrange("p one e -> p (one e)")
              a1 = t1_[:].rearrange("p one e -> p (one e)")
              ea3l = fp.tile([8, EF], F32, tag="ea3l")
              nc.sync.dma_start(ea3l[:], ea3locT[:])
              fB0 = fp.tile([128, EF], F32, tag="fB0")
              fB1 = fp.tile([128, EF], F32, tag="fB1")
              fB2 = fp.tile([128, EF], F32, tag="fB2")
              fB = [fB0, fB1, fB2]
              nc.vector.tensor_tensor(fB[0][:], a0, a1, op=ALU.add)
              nc.vector.tensor_tensor(fB[1][:], a0, a1, op=ALU.mult)
              nc.vector.tensor_tensor(fB[2][:], a0, a1, op=ALU.subtract)
              nc.vector.tensor_tensor(fB[2][:], fB[2][:], fB[2][:], op=ALU.mult)
              pacc = p1.tile([1, EF], F32, tag="pm")
              for bi in range(3):
                pw = p2.tile([128, EF], F32, tag="p512")
                nc.tensor.matmul(pw[:], c_lwWt[:, bi * 128:(bi + 1) * 128],
                                 ea3l[:], start=True, stop=True)
                pr = fp.tile([128, EF], F32, tag="prod")
                nc.vector.tensor_tensor(pr[:], fB[bi][:], pw[:], op=ALU.mult)
                nc.tensor.matmul(pacc[:], c_ones[:], pr[:],
                                 start=(bi == 0), stop=False)
              nc.tensor.matmul(pacc[:], c_lbp[:], ea3l[:], start=False, stop=True)
              ot = fp.tile([1, EF], F32, tag="ot")
              nc.vector.tensor_copy(ot[:], pacc[:])
              nc.sync.dma_start(out_f[:], ot[:])

    nc.compile()
    return nc


_CACHE = {}


def kernel(**inputs):
    static, in_maps = _host_prep(inputs)
    if static not in _CACHE:
        _CACHE[static] = _build(*static)
    nc = _CACHE[static]
    res = run_bass_kernel_spmd(nc, in_maps, list(range(C))).results
    return np.concatenate(
        [res[c]["out_f"][0, :E3 // C] for c in range(C)]).astype(np.float32)

